# revision 15
# baseline (speedup 1.0000x reference)
"""AttnGCN layer on 8 TRN2 NeuronCores — data-parallel over batch.

Per-core (one sample b):
  prep per 512-row group g (emitted inline at oc0's ib=4g boundary so the
  scheduler overlaps late prep with the oc0 attention pipeline):
    x8 = fp8(x)                                    (ACT/DVE halves)
    xT8 = transpose(x8) via plain matmuls against an fp8 identity
          (streams at full MM rate; transpose-mode won't engage HAM)
    kT/qT = W8^T @ xT8                             (fp8 DoubleRow, per-fc
                                                    single-bank PSUM tiles,
                                                    evacs alternate ACT/DVE)
  bias handling (exact softmax algebra): q·bk and bq·bk are constant per
  softmax row -> cancel; bq·k varies per key i -> a [1,N] PE row
  (bq8^T @ kT), transposed per-partition, folded into the exp bias cb2.
  main loop per o-chunk (512 rows):
    sT[i,o] = k_i . q_o  + C'*eT[i,o]              (mask folded into PSUM via
                                                    lhsT=e-block matmuls against
                                                    a scaled identity)
    pT = exp(alpha*sT + cb2[i])                    (ACT, masked entries -> ~0)
    S[o] = sum_i pT[i,o]                           (ones-vector PE matmul)
    ctxT[e,o] = sum_i x8[i,e] * pT[i,o]            (PE fp8 DR, accum over i)
    out_pre[o,c] = sum_e ctxT[e,o] * Wc[e,c]       (PE bf16)
    x = x + out_pre/S ; LayerNorm(x)*gamma + beta  (rstd = exp(-0.5*ln(var+eps))
                                                    so ACT stays on ONE table set)

Self-contained: hardcodes shapes from the problem spec.
"""

import math
from contextlib import ExitStack

import numpy as np

import concourse.bass as bass
import concourse.tile as tile
from concourse import mybir
from concourse.vector_clock import ScopedClock

F32 = mybir.dt.float32
F32R = mybir.dt.float32r
BF16 = mybir.dt.bfloat16
FP8 = mybir.dt.float8e4

B = 8
N = 2048
D = 512
P = 128
NB = N // P       # 16 i-blocks
EC = D // P       # 4 chunks of the embed/dff dim
OC = N // 512     # 4 o-chunks of 512 attn rows
ALPHA = 1.0 / math.sqrt(D)
CPRIME = 1024.0           # mask scale inside PSUM (exactly representable)
SHIFT = 3.0               # softmax-invariant shift keeping exp() in fp8 range
CBIAS = CPRIME * ALPHA + SHIFT  # subtracted in the exp bias


# ---------------------------------------------------------------------------
# Workaround: walrus CoreV3 rejects >2 sem waits on the TileContext final
# drain ("Too many sync wait commands"). Hoist waits onto preceding nops.
def _patched_drain_and_barrier(self, tick_clock, wait_clock):
    nc = self.nc
    carrier = nc.sync.nop(nofuse=True)
    wait_clock.add_sem_waits(carrier.ins, ScopedClock({None: tick_clock.global_clock}))
    si = carrier.ins.sync_info
    waits = list(si.on_wait) if si and si.on_wait else []
    if len(waits) > 1:
        si.on_wait = waits[:1]
        for w in waits[1:]:
            n2 = nc.sync.nop(nofuse=True)
            n2.ins.sync_info = mybir.SyncInfo(on_wait=[w], on_update=[])
    nc.sync.drain()
    nc.all_engine_barrier()
    assert self.sems is not None
    popped = nc._tile_sem_poison_stack.pop()
    assert popped is self._sem_poison
    nc.clear_and_free_semaphores(list(self.sems.allocated().values()))
    nc.all_engine_barrier()


def _apply_patches():
    tile.TileContext._drain_and_barrier = _patched_drain_and_barrier


def _split_excess_waits(nc, limit=1):
    """walrus CoreV2/V3 codegen rejects instructions with >2 sem waits;
    hoist excess waits onto same-engine no-ops inserted just before."""
    n = 0
    for fn in nc.m.functions:
        for blk in fn.blocks:
            out = []
            changed = False
            for inst in blk.instructions:
                si = inst.sync_info
                waits = list(si.on_wait) if si and si.on_wait else []
                if len(waits) > limit:
                    keep = waits[-limit:]
                    for w in waits[:-limit]:
                        n += 1
                        nop = mybir.InstNoOp(name=f"I-wsplit-{n}", ins=[], outs=[])
                        nop.engine = inst.engine
                        nop.sync_info = mybir.SyncInfo(on_wait=[w], on_update=[])
                        out.append(nop)
                    si.on_wait = keep
                    changed = True
                out.append(inst)
            if changed:
                blk.instructions = out
    return n


def _identity(nc, ap, diag):
    nc.gpsimd.memset(ap, 0.0)
    nc.gpsimd.affine_select(
        out=ap,
        in_=ap,
        compare_op=mybir.AluOpType.not_equal,
        fill=diag,
        base=0,
        pattern=[[-1, ap.shape[0]]],
        channel_multiplier=1,
    )


def build_nc():
    nc = bass.Bass()
    x_ext = nc.declare_dram_parameter("node_fts", [N, D], F32, isOutput=False)
    e_ext = nc.declare_dram_parameter("rel_edges", [N, N], F32, isOutput=False)
    wq_ext = nc.declare_dram_parameter("Wq", [D, D], F32, isOutput=False)
    bq_ext = nc.declare_dram_parameter("bq", [D], F32, isOutput=False)
    wk_ext = nc.declare_dram_parameter("Wk", [D, D], F32, isOutput=False)
    bk_ext = nc.declare_dram_parameter("bk", [D], F32, isOutput=False)
    wc_ext = nc.declare_dram_parameter("Wc", [D, D], F32, isOutput=False)
    g_ext = nc.declare_dram_parameter("gamma", [D], F32, isOutput=False)
    be_ext = nc.declare_dram_parameter("beta", [D], F32, isOutput=False)
    out_ext = nc.declare_dram_parameter("out", [N, D], F32, isOutput=True)

    with tile.TileContext(nc) as tc, ExitStack() as ctx:
        singles = ctx.enter_context(tc.tile_pool(name="singles", bufs=1))
        wstage = ctx.enter_context(tc.tile_pool(name="wstage", bufs=2))
        eoc = ctx.enter_context(tc.tile_pool(name="eoc", bufs=5))
        e16p = ctx.enter_context(tc.tile_pool(name="e16p", bufs=4))
        ptp = ctx.enter_context(tc.tile_pool(name="ptp", bufs=4))
        ctxp = ctx.enter_context(tc.tile_pool(name="ctxp", bufs=2))
        rowp = ctx.enter_context(tc.tile_pool(name="rowp", bufs=3))
        epi = ctx.enter_context(tc.tile_pool(name="epi", bufs=2))
        xsbp = ctx.enter_context(tc.tile_pool(name="xsbp", bufs=5))
        sps = ctx.enter_context(tc.tile_pool(name="sps", bufs=3, space="PSUM"))
        ctxps_pool = ctx.enter_context(tc.tile_pool(name="ctxps", bufs=1, space="PSUM"))
        spsum = ctx.enter_context(tc.tile_pool(name="spsum", bufs=1, space="PSUM"))

        # ---- persistent tiles -------------------------------------------
        xs_tiles = [
            singles.tile([P, 4, D], F32, tag=f"xs{g}", name=f"xs{g}") for g in range(4)
        ]
        x8g = [
            singles.tile([P, 4, D], FP8, tag=f"x8g{g}", name=f"x8g{g}")
            for g in range(4)
        ]
        xt8 = [
            singles.tile([P, EC, 512], FP8, tag=f"xt8{g}", name=f"xt8{g}")
            for g in range(4)
        ]
        qt8 = singles.tile([P, EC, N], FP8, tag="qt8")
        kt8 = singles.tile([P, EC, N], FP8, tag="kt8")
        wq8 = singles.tile([P, EC, D], FP8, tag="wq8")
        wk8 = singles.tile([P, EC, D], FP8, tag="wk8")
        wc16 = singles.tile([P, EC, D], BF16, tag="wc16")
        bqt = singles.tile([P, EC], F32, tag="bqt")
        bq8 = singles.tile([P, EC, 16], FP8, tag="bq8")  # 16-wide: DR step%16==0
        gamma_b = singles.tile([P, D], F32, tag="gamma_b")
        beta_b = singles.tile([P, D], F32, tag="beta_b")
        identf = singles.tile([P, P], F32, tag="identf")
        ident8 = singles.tile([P, P], FP8, tag="ident8")
        maskid16 = singles.tile([P, P], BF16, tag="maskid16")
        ones8 = singles.tile([P, 2, 16], FP8, tag="ones8")
        one32 = singles.tile([1, 1], F32, tag="one32")
        eps_t = singles.tile([P, 1], F32, tag="eps_t")
        zero_t = singles.tile([P, 1], F32, tag="zero_t")
        alpha_t = singles.tile([P, 1], F32, tag="alpha_t")
        cbias2_t = singles.tile([P, 1], F32, tag="cbias2_t")
        cb2 = singles.tile([P, NB], F32, tag="cb2")
        scratch1 = singles.tile([P, 1], F32, tag="scratch1")

        _identity(nc, identf, 1.0)
        _identity(nc, maskid16, CPRIME)
        nc.gpsimd.memset(ones8, 1.0)
        nc.gpsimd.memset(one32, 1.0)
        nc.gpsimd.memset(eps_t, 1e-5)
        nc.gpsimd.memset(zero_t, 0.0)
        nc.gpsimd.memset(alpha_t, ALPHA)
        nc.gpsimd.memset(cbias2_t, -CBIAS)
        nc.vector.tensor_copy(out=ident8, in_=identf)

        # preload the (single) ACT table set during the DMA head
        nc.scalar.activation(
            out=scratch1, in_=eps_t,
            func=mybir.ActivationFunctionType.Exp, bias=zero_t[:, 0:1], scale=1.0,
        )

        # ---- DMA issues --------------------------------------------------
        for g in range(4):
            nc.sync.dma_start(
                out=xs_tiles[g],
                in_=x_ext[g * 4 * P : (g + 1) * 4 * P, :].rearrange(
                    "(ib p) e -> p ib e", p=P
                ),
            )
        w_stage_tiles = []
        for w_ext in (wq_ext, wk_ext):
            ws = wstage.tile([P, EC, D], F32, tag="wstage")
            nc.scalar.dma_start(
                out=ws, in_=w_ext[:, :].rearrange("(ec p) f -> p ec f", p=P)
            )
            w_stage_tiles.append(ws)
        nc.scalar.dma_start(out=bqt, in_=bq_ext[:].rearrange("(fc p) -> p fc", p=P))
        ge = g_ext[:]
        nc.scalar.dma_start(
            out=gamma_b,
            in_=bass.AP(tensor=ge.tensor, offset=ge.offset, ap=[[0, P], *ge.ap]),
        )
        bea = be_ext[:]
        nc.scalar.dma_start(
            out=beta_b,
            in_=bass.AP(tensor=bea.tensor, offset=bea.offset, ap=[[0, P], *bea.ap]),
        )

        # edge loader: one quarter of an o-chunk's mask columns at a time
        def emit_e_quarter(oc, q, engine="vector"):
            ef = eoc.tile([P, 4, 512], F32, tag="ef", name=f"ef{oc}{q}")
            nc.sync.dma_start(
                out=ef,
                in_=e_ext[
                    oc * 512 : (oc + 1) * 512, q * 512 : (q + 1) * 512
                ].rearrange("(s p) f -> p s f", p=P),
            )
            e16 = e16p.tile([P, 4, 512], BF16, tag="e16", name=f"e16{oc}{q}")
            # f32->bf16 cast off the ACT exp critical path
            if engine == "gpsimd":
                nc.gpsimd.tensor_copy(out=e16, in_=ef)
            else:
                nc.vector.tensor_copy(out=e16, in_=ef)
            return e16

        # oc0 q0/q1: DMA early, cast on (otherwise idle) gpsimd during prep
        e16_pre = [emit_e_quarter(0, q, engine="gpsimd") for q in range(2)]

        # ---- HAM warmup: dummy matmul burst while the first DMAs land ----
        warm_ps = sps.tile([P, 512], F32, tag="sps")
        for j in range(56):
            nc.tensor.matmul(
                out=warm_ps[:, (j % 4) * P : (j % 4 + 1) * P],
                lhsT=maskid16,
                rhs=maskid16,
                start=True,
                stop=True,
                skip_group_check=True,
            )

        # ---- weight casts (fp8 for DR projections) ----------------------
        nc.scalar.copy(out=wq8, in_=w_stage_tiles[0])
        nc.vector.tensor_copy(out=wk8, in_=w_stage_tiles[1])
        nc.vector.tensor_copy(out=bq8[:, :, 0], in_=bqt)

        # deferred Wc staging (first used at the oc0 tail)
        ws = wstage.tile([P, EC, D], F32, tag="wstage")
        nc.scalar.dma_start(
            out=ws, in_=wc_ext[:, :].rearrange("(ec p) f -> p ec f", p=P)
        )
        nc.vector.tensor_copy(out=wc16, in_=ws)

        # ---- per-group prep: k/q projections + exp-bias chunk -----------
        # Emitted inline at oc0's ib=4g boundary; all PSUM flows through the
        # single-bank sps ring so oc0's ctx accumulator can coexist.
        def prep_g(g):
            xs = xs_tiles[g]
            nc.scalar.copy(out=x8g[g][:, 0:2, :], in_=xs[:, 0:2, :])
            nc.vector.tensor_copy(out=x8g[g][:, 2:4, :], in_=xs[:, 2:4, :])
            # transposes as plain matmuls against the fp8 identity
            for ec in range(EC):
                tp = sps.tile([P, 512], F32, tag="sps", name=f"tp{g}{ec}")
                for k4 in range(4):
                    nc.tensor.matmul(
                        out=tp[:, k4 * P : (k4 + 1) * P],
                        lhsT=x8g[g][:, k4, ec * P : (ec + 1) * P],
                        rhs=ident8,
                        start=(k4 == 0),
                        stop=(k4 == 3),
                        skip_group_check=True,
                    )
                if ec % 2 == 0:
                    nc.scalar.copy(out=xt8[g][:, ec, :], in_=tp)
                else:
                    nc.vector.tensor_copy(out=xt8[g][:, ec, :], in_=tp)
            # projections (no bias): fp8 DoubleRow, k first (oc0 needs kT)
            for w8, dst in ((wk8, kt8), (wq8, qt8)):
                for fc in range(EC):
                    ps = sps.tile([P, 512], F32, tag="sps", name=f"pj{g}{fc}")
                    for dc in (0, 2):
                        nc.tensor.matmul(
                            out=ps,
                            lhsT=w8[:, dc : dc + 2, fc * P : (fc + 1) * P],
                            rhs=xt8[g][:, dc : dc + 2, :],
                            start=(dc == 0),
                            stop=(dc == 2),
                            perf_mode=mybir.MatmulPerfMode.DoubleRow,
                            skip_group_check=True,
                        )
                    if fc % 2 == 0:
                        nc.scalar.copy(
                            out=dst[:, fc, g * 512 : (g + 1) * 512], in_=ps
                        )
                    else:
                        nc.vector.tensor_copy(
                            out=dst[:, fc, g * 512 : (g + 1) * 512], in_=ps
                        )
            # exp bias chunk: cb2[i] = ALPHA*(bq . k_i) - CBIAS for i in g
            row_ps = sps.tile([P, 512], F32, tag="sps", name=f"row{g}")
            for dc in (0, 2):
                nc.tensor.matmul(
                    out=row_ps[0:1, :],
                    lhsT=bq8[:, dc : dc + 2, 0:1],
                    rhs=kt8[:, dc : dc + 2, g * 512 : (g + 1) * 512],
                    start=(dc == 0),
                    stop=(dc == 2),
                    perf_mode=mybir.MatmulPerfMode.DoubleRow,
                    skip_group_check=True,
                )
            row_sb = rowp.tile([1, 512], F32, tag="row_sb")
            nc.vector.tensor_copy(out=row_sb, in_=row_ps[0:1, :])
            cb_psg = sps.tile([P, 4], F32, tag="sps", name=f"cb{g}")
            for j in range(4):
                nc.tensor.matmul(
                    out=cb_psg[:, j : j + 1],
                    lhsT=row_sb[0:1, j * P : (j + 1) * P],
                    rhs=one32,
                    is_transpose=True,
                    start=(j == 0),
                    stop=(j == 3),
                    skip_group_check=True,
                )
            nc.vector.tensor_scalar(
                cb2[:, g * 4 : (g + 1) * 4],
                cb_psg,
                alpha_t[:, 0:1],
                cbias2_t[:, 0:1],
                mybir.AluOpType.mult,
                mybir.AluOpType.add,
            )

        # ---- main loop over o-chunks ------------------------------------
        for oc in range(OC):
            if oc == 0:
                e16_q = e16_pre + [emit_e_quarter(0, q) for q in range(2, 4)]
            else:
                e16_q = [emit_e_quarter(oc, q) for q in range(4)]

            ctx_ps = ctxps_pool.tile([P, EC, 512], F32, tag="ctxps")
            s_ps = spsum.tile([1, 512], F32, tag="spsum")

            pt2 = None
            for ib in range(NB):
                if oc == 0 and ib % 4 == 0:
                    prep_g(ib // 4)
                e16, il = e16_q[ib // 4], ib % 4
                sp = sps.tile([P, 512], F32, tag="sps")
                for s in range(4):
                    # start=True clears the whole PSUM bank -> only on s==0;
                    # later mask MMs hit has_written=0 and write directly.
                    nc.tensor.matmul(
                        out=sp[:, s * P : (s + 1) * P],
                        lhsT=e16[:, s, il * P : (il + 1) * P],
                        rhs=maskid16,
                        start=(s == 0),
                        stop=False,
                        skip_group_check=True,
                    )
                for dc in (0, 2):
                    nc.tensor.matmul(
                        out=sp,
                        lhsT=kt8[:, dc : dc + 2, ib * P : (ib + 1) * P],
                        rhs=qt8[:, dc : dc + 2, oc * 512 : (oc + 1) * 512],
                        start=False,
                        stop=(dc == 2),
                        perf_mode=mybir.MatmulPerfMode.DoubleRow,
                        skip_group_check=True,
                    )
                if ib % 2 == 0:
                    pt2 = ptp.tile([P, 2, 512], FP8, tag="pt")
                nc.scalar.activation(
                    out=pt2[:, ib % 2, :],
                    in_=sp,
                    func=mybir.ActivationFunctionType.Exp,
                    bias=cb2[:, ib : ib + 1],
                    scale=ALPHA,
                )
                if ib % 2 == 1:
                    j = (ib % 4) - 1
                    for ec in range(EC):
                        nc.tensor.matmul(
                            out=ctx_ps[:, ec, :],
                            lhsT=x8g[ib // 4][:, j : j + 2, ec * P : (ec + 1) * P],
                            rhs=pt2,
                            start=(ib == 1),
                            stop=(ib == NB - 1),
                            perf_mode=mybir.MatmulPerfMode.DoubleRow,
                            skip_group_check=True,
                        )
                    nc.tensor.matmul(
                        out=s_ps,
                        lhsT=ones8[:, :, 0:1],
                        rhs=pt2,
                        start=(ib == 1),
                        stop=(ib == NB - 1),
                        perf_mode=mybir.MatmulPerfMode.DoubleRow,
                        skip_group_check=True,
                    )

            # unnormalized ctx -> SBUF bf16, split ACT/DVE to cut latency
            ctx16 = ctxp.tile([P, EC, 512], BF16, tag="ctx16")
            for ec in range(EC):
                if ec % 2 == 0:
                    nc.scalar.copy(out=ctx16[:, ec, :], in_=ctx_ps[:, ec, :])
                else:
                    nc.vector.tensor_copy(out=ctx16[:, ec, :], in_=ctx_ps[:, ec, :])

            s_sb = rowp.tile([1, 512], F32, tag="s_sb")
            nc.vector.tensor_copy(out=s_sb, in_=s_ps)

            # out_pre = ctx_unnorm @ Wc ; scale rows by 1/S ; residual + LN
            x_tiles = []
            mv4 = epi.tile([P, 4, 2], F32, tag="mv4")
            s_col = None
            rs_col = None
            for os4 in range(4):
                opre = sps.tile([P, 512], F32, tag="sps")
                for ec in range(EC):
                    nc.tensor.matmul(
                        out=opre,
                        lhsT=ctx16[:, ec, os4 * P : (os4 + 1) * P],
                        rhs=wc16[:, ec, :],
                        start=(ec == 0),
                        stop=(ec == EC - 1),
                        skip_group_check=True,
                    )
                if os4 == 0:
                    # 1/S per-partition: S row -> PE transpose -> recip
                    s_col = sps.tile([P, 4], F32, tag="sps")
                    for j in range(4):
                        nc.tensor.matmul(
                            out=s_col[:, j : j + 1],
                            lhsT=s_sb[0:1, j * P : (j + 1) * P],
                            rhs=one32,
                            is_transpose=True,
                            start=(j == 0),
                            stop=(j == 3),
                            skip_group_check=True,
                        )
                    rs_col = rowp.tile([P, 4], F32, tag="rs_col")
                    nc.vector.reciprocal(out=rs_col, in_=s_col)
                t0 = epi.tile([P, D], F32, tag="t0")
                nc.scalar.mul(t0, opre, rs_col[:, os4 : os4 + 1])
                x_sb = xsbp.tile([P, D], F32, tag="x_sb")
                nc.vector.tensor_add(x_sb, t0, xs_tiles[oc][:, os4, :])
                x_tiles.append(x_sb)
                stats = epi.tile([P, 6], F32, tag="stats")
                nc.vector.bn_stats(out=stats, in_=x_sb)
                nc.vector.bn_aggr(out=mv4[:, os4, :], in_=stats)
            # rstd = exp(-0.5*ln(var+eps)) -- stays on the exp/ln table set
            ln4 = epi.tile([P, 4], F32, tag="ln4")
            nc.scalar.activation(
                out=ln4,
                in_=mv4[:, :, 1],
                func=mybir.ActivationFunctionType.Ln,
                bias=eps_t[:, 0:1],
                scale=1.0,
            )
            rs4 = epi.tile([P, 4], F32, tag="rs4")
            nc.scalar.activation(
                out=rs4,
                in_=ln4,
                func=mybir.ActivationFunctionType.Exp,
                bias=zero_t[:, 0:1],
                scale=-0.5,
            )
            # pass 2: normalize (DVE), gamma (gpsimd; DVE on the final chunk
            # where the serial epilogue tail is latency-critical), beta (DVE)
            for os4 in range(4):
                t_sb = epi.tile([P, D], F32, tag="t_sb")
                nc.vector.tensor_scalar(
                    t_sb,
                    x_tiles[os4],
                    mv4[:, os4, 0:1],
                    rs4[:, os4 : os4 + 1],
                    mybir.AluOpType.subtract,
                    mybir.AluOpType.mult,
                )
                g_sb = epi.tile([P, D], F32, tag="g_sb")
                if oc == OC - 1:
                    nc.vector.tensor_mul(g_sb, t_sb, gamma_b)
                else:
                    nc.gpsimd.tensor_mul(g_sb, t_sb, gamma_b)
                o_sb = epi.tile([P, D], F32, tag="o_sb")
                nc.vector.tensor_add(o_sb, g_sb, beta_b)
                r0 = (oc * 4 + os4) * P
                nc.sync.dma_start(out=out_ext[r0 : r0 + P, :], in_=o_sb)

    _ = bk_ext  # bk only enters scores via per-softmax-row constants -> cancels
    _split_excess_waits(nc)
    return nc


_NC_CACHE = None


def kernel(**inputs) -> np.ndarray:
    global _NC_CACHE
    _apply_patches()
    from concourse.bass_utils import run_bass_kernel_spmd

    node_fts = np.ascontiguousarray(np.asarray(inputs["node_fts"], dtype=np.float32))
    rel_edges = np.ascontiguousarray(np.asarray(inputs["rel_edges"], dtype=np.float32))
    shared = {
        k: np.ascontiguousarray(np.asarray(inputs[k], dtype=np.float32))
        for k in ("Wq", "bq", "Wk", "bk", "Wc", "gamma", "beta")
    }
    if _NC_CACHE is None:
        _NC_CACHE = build_nc()
    in_maps = [
        {"node_fts": node_fts[b], "rel_edges": rel_edges[b], **shared}
        for b in range(B)
    ]
    res = run_bass_kernel_spmd(_NC_CACHE, in_maps, core_ids=list(range(B)))
    return np.stack([res.results[b]["out"] for b in range(B)]).astype(np.float32)
